# revision 5
# baseline (speedup 1.0000x reference)
"""TRN2 Bass kernel for nn_ConceptEmbeddingConceptPred.

Computes y = concat([einsum('bjd,ijd->bi', x, W_emb) + b_loo,
                     einsum('bjd,hjd->bh', x, W_full) + b_full], axis=1)
where W_emb is the leave-one-out scatter-embedding of W_loo (zero diagonal).

Flattened, this is a (4096 x 16384) @ (16384 x 136) GEMM.

Distribution: contraction(k)-parallel over the 8 cores — core c owns
concepts j in [16c, 16c+16) (k-slice of 2048). Each core computes a full
(136, 4096) partial product; partials are summed on the host (cheap),
bias added, transposed, concatenated.

v6 dataflow (fp8e3 x stream, fp16 weights, fp32 PSUM accumulate):
  - x is transposed and cast to float8_e3m4 on the host (1.3% rel rms
    quantization, ~halves DMA bytes to 8.4 MB/core). The PE accepts the
    mixed-dtype matmul (fp16 stationary x fp8e3 moving) natively at the
    same 1 col/cycle stream rate.
  - all 16 x k-tiles stay resident in SBUF ([128, 4096] fp8 = 4 KB per
    partition each); DMA'd once in round halves for early starts.
  - phase-separated passes to avoid PE tile-mode-switch drains:
    loo round 0 (64 back-to-back M=128 matmuls), loo round 1, then one
    full-probe pass (M=8) over all 8 batch chunks per k-tile using
    3-way col-group concurrency (q0/q32/q64; quadrant 3 is unusable):
    per k-tile the 8 chunk streams fold into 3 waves instead of 8.
  - one large output DMA per loo round + 3 packed full-probe bank DMAs;
    epilogue copies split across vector/scalar engines.
"""

import sys

for _p in ("/opt/trn_rl_repo",):
    if _p not in sys.path:
        sys.path.append(_p)

import numpy as np
import ml_dtypes
import concourse.bacc as bacc
import concourse.mybir as mybir
import concourse.tile as tile
from concourse.bass_utils import run_bass_kernel_spmd

dt = mybir.dt

B, C, D, H = 4096, 128, 128, 8
NCORES = 8
JPC = C // NCORES  # 16 concept (= k) tiles per core
KPC = JPC * D  # 2048 contraction elements per core
BCHUNK = 512  # batch per PSUM accumulation chunk (fp32 bank limit)
NCH = 8  # 512-col chunks across the full batch
RCHUNK = 2048  # batch cols per loo round
NR = 2  # loo rounds
NWARM = 40  # dummy 128-col matmuls to ramp the PE p-state during DMA fill
NFB = 3  # full-probe PSUM banks (8 chunks packed 3+3+2 across col groups)

_nc_cache = None


def _build():
    global _nc_cache
    if _nc_cache is not None:
        return _nc_cache

    nc = bacc.Bacc(
        "TRN2", target_bir_lowering=False, debug=False, num_devices=NCORES
    )
    xt_d = nc.dram_tensor("x_t", (KPC, B), dt.float8e3, kind="ExternalInput").ap()
    wl_d = nc.dram_tensor(
        "w_loo_t", (D, JPC, C), dt.float16, kind="ExternalInput"
    ).ap()
    wf_d = nc.dram_tensor(
        "w_full_t", (D, JPC, H), dt.float16, kind="ExternalInput"
    ).ap()
    yl_d = nc.dram_tensor("y_loo_t", (C, B), dt.float16, kind="ExternalOutput").ap()
    # full-probe outputs in packed col-group layout: bank w rows
    # [32g : 32g+8] hold chunk c = 3w + g (chunks 0..7 over 512-col chunks)
    yf_d = nc.dram_tensor(
        "y_full_p", (NFB, 128, BCHUNK), dt.float16, kind="ExternalOutput"
    ).ap()

    with tile.TileContext(nc) as tc:
        with (
            tc.tile_pool(name="wpool", bufs=1) as wpool,
            tc.tile_pool(name="xpool", bufs=16) as xpool,
            tc.tile_pool(name="ylpool", bufs=2) as ylpool,
            tc.tile_pool(name="yfpool", bufs=3) as yfpool,
            tc.tile_pool(name="psl", bufs=5, space="PSUM") as psl,
            tc.tile_pool(name="psf", bufs=3, space="PSUM") as psf,
        ):
            wl = wpool.tile([D, JPC, C], dt.float16)
            wf = wpool.tile([D, JPC, H], dt.float16)
            # scalar queue: weights first (wf 4 KB, wl 512 KB in halves)
            nc.scalar.dma_start(wf[:], wf_d[:])
            nc.scalar.dma_start(wl[:, : JPC // 2, :], wl_d[:, : JPC // 2, :])
            nc.scalar.dma_start(wl[:, JPC // 2 :, :], wl_d[:, JPC // 2 :, :])

            # PE p-state warmup in 128x128 tile mode (matches the loo pass,
            # so no mode-switch drain into the first real matmul)
            warm_w = wpool.tile([128, 128], dt.float16)
            warm_x = wpool.tile([128, 128], dt.float16)
            nc.vector.memset(warm_w[:], 0.25)
            nc.vector.memset(warm_x[:], 0.25)
            warm_ps = psl.tile([128, 128], dt.float32, tag="accl", name="warm")
            for _ in range(NWARM):
                nc.tensor.matmul(
                    warm_ps[:], warm_w[:], warm_x[:], start=True, stop=True
                )

            # x tiles: one [128, B] fp8 tile per kt, all resident.
            # Round-0 halves first (kt0 in quarters for fastest first MM),
            # then round-1 halves. Alternate HWDGE queues per kt.
            xts = []
            for kt in range(JPC):
                xts.append(
                    xpool.tile([128, B], dt.float8e3, tag="xn", name=f"xn_{kt}")
                )
            for kt in range(JPC):
                eng = nc.sync if kt % 2 == 0 else nc.scalar
                row = xt_d[kt * 128 : (kt + 1) * 128, :]
                if kt == 0:
                    edges = [0, 512, 1024, 1536, 2048]
                    for a, b in zip(edges, edges[1:]):
                        eng.dma_start(xts[kt][:, a:b], row[:, a:b])
                else:
                    eng.dma_start(xts[kt][:, :RCHUNK], row[:, :RCHUNK])
            for kt in range(JPC):
                eng = nc.sync if kt % 2 == 0 else nc.scalar
                row = xt_d[kt * 128 : (kt + 1) * 128, :]
                eng.dma_start(xts[kt][:, RCHUNK:], row[:, RCHUNK:])

            # loo rounds: 64 back-to-back M=128 matmuls each
            for r in range(NR):
                accs = [
                    psl.tile(
                        [C, BCHUNK], dt.float32, tag="accl", name=f"accl{r}_{c}"
                    )
                    for c in range(4)
                ]
                for kt in range(JPC):
                    for c in range(4):
                        bc = r * 4 + c
                        nc.tensor.matmul(
                            accs[c][:],
                            wl[:, kt, :],
                            xts[kt][:, bc * BCHUNK : (bc + 1) * BCHUNK],
                            start=(kt == 0),
                            stop=(kt == JPC - 1),
                        )
                yl_sb = ylpool.tile([C, RCHUNK], dt.float16, tag="yl")
                for c in range(4):
                    if r == NR - 1 and c % 2 == 1:
                        nc.scalar.copy(
                            yl_sb[:, c * BCHUNK : (c + 1) * BCHUNK], accs[c][:]
                        )
                    else:
                        nc.vector.tensor_copy(
                            yl_sb[:, c * BCHUNK : (c + 1) * BCHUNK], accs[c][:]
                        )
                oeng = nc.sync if r == 0 else nc.scalar
                oeng.dma_start(yl_d[:, r * RCHUNK : (r + 1) * RCHUNK], yl_sb[:])

            # full-probe pass: M=8 matmuls, 8 chunks folded onto col groups
            # q0/q32/q64 (3 concurrent streams); chunk c -> bank c//3, group
            # c%3. Per kt the queue per group is <=3 deep -> 3 waves.
            fbanks = [
                psf.tile([128, BCHUNK], dt.float32, tag="accf", name=f"fb{w}")
                for w in range(NFB)
            ]
            for kt in range(JPC):
                for c in range(NCH):
                    w, g = divmod(c, NFB)
                    nc.tensor.matmul(
                        fbanks[w][32 * g : 32 * g + H, :],
                        wf[:, kt, :],
                        xts[kt][:, c * BCHUNK : (c + 1) * BCHUNK],
                        start=(kt == 0),
                        stop=(kt == JPC - 1),
                    )
            for w in range(NFB):
                yf_sb = yfpool.tile([128, BCHUNK], dt.float16, tag="yf")
                if w == 1:
                    nc.scalar.copy(yf_sb[:], fbanks[w][:])
                else:
                    nc.vector.tensor_copy(yf_sb[:], fbanks[w][:])
                oeng = nc.sync if w % 2 == 0 else nc.scalar
                oeng.dma_start(yf_d[w], yf_sb[:])

    nc.compile()
    _nc_cache = nc
    return nc


def _embed_loo_weights(W_loo):
    # probe i sees concepts j != i; scatter into (C, C, D) with zero row at j=i
    I = np.arange(C)[:, None]
    J = np.arange(C)[None, :]
    src = np.clip(J - (J > I).astype(np.int64), 0, C - 2)  # (C, C)
    W_emb = np.take_along_axis(W_loo, src[:, :, None], axis=1)  # (C, C, D)
    return W_emb * (J != I)[:, :, None].astype(W_loo.dtype)


def _prep_in_maps(x, W_loo, W_full):
    x32 = np.asarray(x, dtype=np.float32)
    # (C, D, B) so each core's (JPC, D, B) k-slice is a contiguous view
    xt_all = np.ascontiguousarray(x32.transpose(1, 2, 0)).astype(
        ml_dtypes.float8_e3m4
    )
    W_emb = _embed_loo_weights(np.asarray(W_loo, dtype=np.float32))
    W_full = np.asarray(W_full, dtype=np.float32)
    in_maps = []
    for c in range(NCORES):
        jsl = slice(c * JPC, (c + 1) * JPC)
        xt_c = xt_all[jsl].reshape(KPC, B)
        # stationary layouts: [d, kt, out] so K (=d) is the partition dim
        wl_c = np.ascontiguousarray(
            W_emb[:, jsl, :].transpose(2, 1, 0).astype(np.float16)
        )
        wf_c = np.ascontiguousarray(
            W_full[:, jsl, :].transpose(2, 1, 0).astype(np.float16)
        )
        in_maps.append({"x_t": xt_c, "w_loo_t": wl_c, "w_full_t": wf_c})
    return in_maps


def _assemble(results, b_loo, b_full):
    y_loo_t = np.zeros((C, B), np.float64)
    y_full_t = np.zeros((H, B), np.float64)
    for res in results:
        y_loo_t += res["y_loo_t"]
        yf_p = res["y_full_p"]  # (NFB, 128, BCHUNK) packed col groups
        for c in range(NCH):
            w, g = divmod(c, NFB)
            y_full_t[:, c * BCHUNK : (c + 1) * BCHUNK] += yf_p[
                w, 32 * g : 32 * g + H, :
            ]
    y_loo = (y_loo_t.T + np.asarray(b_loo, np.float64)[None, :]).astype(np.float32)
    y_full = (y_full_t.T + np.asarray(b_full, np.float64)[None, :]).astype(np.float32)
    return np.concatenate([y_loo, y_full], axis=1)


def run_spmd(x, W_loo, b_loo, W_full, b_full, trace=False):
    nc = _build()
    in_maps = _prep_in_maps(x, W_loo, W_full)
    res = run_bass_kernel_spmd(
        nc, in_maps, core_ids=list(range(NCORES)), trace=trace
    )
    return _assemble(res.results, b_loo, b_full), res


def kernel(x, W_loo, b_loo, W_full, b_full):
    out, _ = run_spmd(x, W_loo, b_loo, W_full, b_full)
    return out
